# revision 7
# baseline (speedup 1.0000x reference)
"""Trainium2 Bass kernel for nn_ConvFlow (VITS ConvFlow: pre 1x1 conv ->
DDSConv x3 -> proj 1x1 conv -> rational-quadratic spline).

Data-parallel: batch 16 split 2-per-core across 8 NeuronCores; parameters
replicated.  kernel(**inputs) takes full unsharded inputs and returns
(x_out [16,192,1024] f32, logdet [16] f32) matching reference().

Math notes (verified against the reference formulas):
  * proj weights reordered host-side from row order c*29+j to j*96+c so each
    spline parameter plane j is a contiguous [96, T] matmul output; the
    1/sqrt(FILT) scale for width/height planes and proj_b are folded into the
    weights / activation bias.
  * softmax+cumsum+searchsorted restructured: with e'_j = 10C*exp(uw_j),
    partial sums F'_k, S = sum_j exp(uw_j), P = 0.01*S, the reference edge
    test xc >= edge_k is exactly (xc+TB)*S >= F'_k + k*P, and
    theta = ((xc+TB)*S - sel(F_k)) / (P + sel(e'_{k+1})).  All selected
    quantities come from copy_predicated chains over the monotone masks.
  * x_mask is all-ones per the input spec; masking multiplies are identity
    and are skipped.
"""

import sys

for _p in ("/opt/trn_rl_repo", "/root/.axon_site/_ro/trn_rl_repo"):
    if _p not in sys.path:
        sys.path.insert(0, _p)

import contextlib

import numpy as np

import concourse.bacc as bacc
import concourse.mybir as mybir
import concourse.tile as tile
from concourse.bass_utils import run_bass_kernel_spmd

dt = mybir.dt
AF = mybir.ActivationFunctionType
ALU = mybir.AluOpType

# ---- problem constants (hardcoded per spec) ----
B, IN_CH, T = 16, 192, 1024
HALF = IN_CH // 2          # 96
FILT = 256
KW = 3
NL = 3
NB = 10
TB = 5.0
MINW = 1e-3
MIND = 1e-3
NPAR = 3 * NB - 1          # 29
INV_SQRT_FILT = float(1.0 / np.sqrt(FILT))
CC = 1.0 - MINW * NB
LN10C = float(np.log(10.0 * CC))
NCORES = 8
BLOC = B // NCORES         # 2
FREE = BLOC * T            # 2048
NCH = 4
CH = FREE // NCH           # 512
EPS = 1e-5

_CACHE = {}


def _build_program():
    nc = bacc.Bacc("TRN2", target_bir_lowering=False, debug=False)

    f32, f32r, u8 = dt.float32, dt.float32r, dt.uint8

    # ---------------- DRAM I/O ----------------
    x_d = nc.dram_tensor("x", [BLOC, IN_CH, T], f32, kind="ExternalInput")
    wpre_d = nc.dram_tensor("wpre", [HALF, FILT], f32, kind="ExternalInput")
    wc11_d = nc.dram_tensor("wc11", [NL, FILT, FILT], f32, kind="ExternalInput")
    wproj_d = nc.dram_tensor("wproj", [FILT, NPAR * HALF], f32, kind="ExternalInput")
    sepw_d = nc.dram_tensor("sepw", [NL, KW, FILT], f32, kind="ExternalInput")
    sepb_d = nc.dram_tensor("sepb", [NL, FILT], f32, kind="ExternalInput")
    n1g_d = nc.dram_tensor("n1g", [NL, FILT], f32, kind="ExternalInput")
    n1b_d = nc.dram_tensor("n1b", [NL, FILT], f32, kind="ExternalInput")
    n2g_d = nc.dram_tensor("n2g", [NL, FILT], f32, kind="ExternalInput")
    n2b_d = nc.dram_tensor("n2b", [NL, FILT], f32, kind="ExternalInput")
    c11b_d = nc.dram_tensor("c11b", [NL, FILT], f32, kind="ExternalInput")
    preb_d = nc.dram_tensor("preb", [FILT], f32, kind="ExternalInput")
    pjb_d = nc.dram_tensor("pjb", [NPAR, HALF], f32, kind="ExternalInput")
    bm_d = nc.dram_tensor("bmeans", [2, NL], f32, kind="ExternalInput")

    xout_d = nc.dram_tensor("xout", [BLOC, IN_CH, T], f32, kind="ExternalOutput")
    ld_d = nc.dram_tensor("ld", [1, BLOC], f32, kind="ExternalOutput")

    NSUB = FREE // 512

    with tile.TileContext(nc) as tc:
        with contextlib.ExitStack() as ctx:
            wp = ctx.enter_context(tc.tile_pool(name="weights", bufs=1))
            cp = ctx.enter_context(tc.tile_pool(name="consts", bufs=1))
            hp = ctx.enter_context(tc.tile_pool(name="resid", bufs=1))

            # ------------- constants -------------
            one_c = cp.tile([128, 1], f32, tag="onec")
            nc.vector.memset(one_c[:], 1.0)
            eps_c = cp.tile([128, 1], f32, tag="epsc")
            nc.vector.memset(eps_c[:], EPS)
            ones_r = cp.tile([128, 1], f32r, tag="onesr")
            nc.vector.tensor_copy(ones_r[:], one_c[:])
            colones_f = cp.tile([1, 128], f32, tag="conesf")
            nc.vector.memset(colones_f[:], 1.0)
            colones_r = cp.tile([1, 128], f32r, tag="conesr")
            nc.vector.tensor_copy(colones_r[:], colones_f[:])
            czero_f = cp.tile([HALF, CH], f32, tag="czf")
            nc.gpsimd.memset(czero_f[:], 0.0)
            czero_r = cp.tile([HALF, CH], f32r, tag="czr")
            nc.vector.tensor_copy(czero_r[:], czero_f[:])
            cone_d = cp.tile([HALF, CH], f32, tag="coned")
            nc.gpsimd.memset(cone_d[:], 1.0 - MIND)
            ldsb = cp.tile([1, FREE], f32, tag="ldsb")

            def load_cols(dram_ap, tag):
                t_ = cp.tile([128, 2], f32, tag=tag)
                nc.sync.dma_start(t_[:], dram_ap.rearrange("(k p) -> p k", p=128))
                return t_

            sepw_c = [[load_cols(sepw_d[i, t_], f"sepw{i}{t_}") for t_ in range(KW)]
                      for i in range(NL)]
            sepb_c = [load_cols(sepb_d[i], f"sepb{i}") for i in range(NL)]
            n1g_c = [load_cols(n1g_d[i], f"n1g{i}") for i in range(NL)]
            n1b_c = [load_cols(n1b_d[i], f"n1b{i}") for i in range(NL)]
            n2g_c = [load_cols(n2g_d[i], f"n2g{i}") for i in range(NL)]
            n2b_c = [load_cols(n2b_d[i], f"n2b{i}") for i in range(NL)]
            c11b_c = [load_cols(c11b_d[i], f"c11b{i}") for i in range(NL)]
            preb_c = load_cols(preb_d, "preb")
            bm_c = cp.tile([1, 2 * NL], f32, tag="bmc")
            nc.sync.dma_start(bm_c[:], bm_d.rearrange("a b -> (a b)")[None, :])
            pjb_c = cp.tile([HALF, NPAR], f32, tag="pjb")
            nc.sync.dma_start(pjb_c[:], pjb_d.rearrange("j c -> c j"))

            # persistent big tensors
            x1_f = wp.tile([HALF, BLOC, T], f32, tag="x1f")
            nc.sync.dma_start(x1_f[:], x_d[:, HALF:IN_CH, :].rearrange("b c t -> c b t"))
            wpre_r = wp.tile([HALF, FILT], f32r, tag="wpre_r")
            wc11_r = [[wp.tile([128, FILT], f32r, tag=f"wc11_{i}_{k}",
                               name=f"wc11r_{i}_{k}")
                       for k in range(2)] for i in range(NL)]
            wproj_r = [wp.tile([128, NPAR * HALF], f32r, tag=f"wproj_{k}",
                               name=f"wprojr_{k}")
                       for k in range(2)]
            h_t = [hp.tile([128, FREE], f32r, tag=f"h{m}", name=f"ht_{m}")
                   for m in range(2)]

            # x0 passthrough
            nc.sync.dma_start(xout_d[:, 0:HALF, :], x_d[:, 0:HALF, :])

            # ================= conv phase =================
            with tc.tile_pool(name="conv", bufs=12) as cs, \
                 tc.tile_pool(name="cps", bufs=2, space="PSUM") as pool_ps:

                _ctr = [0]

                def cst(shape, dty=f32):
                    _ctr[0] += 1
                    return cs.tile(shape, dty, tag="cs", name=f"cs{_ctr[0]}")

                # weight staging + f32r casts
                wf = cst([HALF, FILT])
                nc.sync.dma_start(wf[:], wpre_d[:])
                nc.vector.tensor_copy(wpre_r[:], wf[:])
                for i in range(NL):
                    for k in range(2):
                        wf = cst([128, FILT])
                        nc.sync.dma_start(wf[:], wc11_d[i, k * 128:(k + 1) * 128, :])
                        nc.vector.tensor_copy(wc11_r[i][k][:], wf[:])
                HP = NPAR * HALF // 2
                for k in range(2):
                    for hh in range(2):
                        wf = cst([128, HP])
                        nc.sync.dma_start(
                            wf[:], wproj_d[k * 128:(k + 1) * 128,
                                           hh * HP:(hh + 1) * HP])
                        nc.vector.tensor_copy(
                            wproj_r[k][:, hh * HP:(hh + 1) * HP], wf[:])

                # x0
                x0_f = cst([HALF, BLOC, T])
                nc.sync.dma_start(x0_f[:],
                                  x_d[:, 0:HALF, :].rearrange("b c t -> c b t"))
                x0_r = cst([HALF, FREE], f32r)
                nc.vector.tensor_copy(x0_r[:], x0_f[:].rearrange("c b t -> c (b t)"))

                # pre 1x1 conv
                for mo in range(2):
                    pm = pool_ps.tile([128, FREE], f32, tag="bigps")
                    for n in range(NSUB):
                        sl = slice(n * 512, (n + 1) * 512)
                        nc.tensor.matmul(pm[:, sl],
                                         wpre_r[:, mo * 128:(mo + 1) * 128],
                                         x0_r[:, sl], start=True, stop=True)
                    nc.scalar.activation(h_t[mo][:], pm[:], AF.Identity,
                                         bias=preb_c[:, mo:mo + 1], scale=1.0)

                def layer_norm_gelu(a_t, bias_c, bmean_ap, gam_c, bet_c, out_dt):
                    sq_t = []
                    for m in range(2):
                        sq = cst([128, FREE], f32r)
                        nc.scalar.activation(sq[:], a_t[m][:], AF.Square,
                                             bias=bias_c[:, m:m + 1], scale=1.0)
                        sq_t.append(sq)
                    suma = pool_ps.tile([1, FREE], f32, tag="bigps")
                    sumq = pool_ps.tile([1, FREE], f32, tag="bigps")
                    for n in range(NSUB):
                        sl = slice(n * 512, (n + 1) * 512)
                        for m in range(2):
                            nc.tensor.matmul(suma[:, sl], ones_r[:], a_t[m][:, sl],
                                             start=(m == 0), stop=(m == 1))
                            nc.tensor.matmul(sumq[:, sl], ones_r[:], sq_t[m][:, sl],
                                             start=(m == 0), stop=(m == 1))
                    mrow = cst([1, FREE], f32r)
                    nc.vector.tensor_scalar(out=mrow[:], in0=suma[:],
                                            scalar1=1.0 / FILT, scalar2=bmean_ap,
                                            op0=ALU.mult, op1=ALU.add)
                    msq = cst([1, FREE])
                    nc.scalar.activation(msq[:], mrow[:], AF.Square,
                                         bias=0.0, scale=1.0)
                    nc.vector.scalar_tensor_tensor(out=msq[:], in0=sumq[:],
                                                   scalar=1.0 / FILT, in1=msq[:],
                                                   op0=ALU.mult, op1=ALU.subtract)
                    srow = cst([1, FREE], f32r)
                    nc.scalar.activation(srow[:], msq[:], AF.Sqrt,
                                         bias=eps_c[:1, :], scale=1.0)
                    mb = pool_ps.tile([128, FREE], f32, tag="bigps")
                    sb = pool_ps.tile([128, FREE], f32, tag="bigps")
                    for n in range(NSUB):
                        sl = slice(n * 512, (n + 1) * 512)
                        nc.tensor.matmul(mb[:, sl], colones_r[:], mrow[:, sl],
                                         start=True, stop=True)
                        nc.tensor.matmul(sb[:, sl], colones_r[:], srow[:, sl],
                                         start=True, stop=True)
                    invb = cst([128, FREE])
                    nc.vector.reciprocal(invb[:], sb[:])
                    out_t = []
                    for m in range(2):
                        z = cst([128, FREE])
                        nc.vector.scalar_tensor_tensor(out=z[:], in0=mb[:],
                                                       scalar=-1.0, in1=a_t[m][:],
                                                       op0=ALU.mult, op1=ALU.add)
                        nc.vector.scalar_tensor_tensor(out=z[:], in0=z[:],
                                                       scalar=bias_c[:, m:m + 1],
                                                       in1=invb[:], op0=ALU.add,
                                                       op1=ALU.mult)
                        o = cst([128, FREE], out_dt)
                        nc.scalar.activation(o[:], z[:], AF.Gelu,
                                             bias=bet_c[:, m:m + 1],
                                             scale=gam_c[:, m:m + 1])
                        out_t.append(o)
                    return out_t

                for i in range(NL):
                    d = KW ** i
                    a_t = []
                    for m in range(2):
                        pad = cs.tile([128, BLOC, T + 2 * d], f32, tag="cs")
                        nc.gpsimd.memset(pad[:, :, 0:d], 0.0)
                        nc.gpsimd.memset(pad[:, :, T + d:T + 2 * d], 0.0)
                        nc.vector.tensor_copy(
                            pad[:, :, d:T + d],
                            h_t[m][:].rearrange("c (b t) -> c b t", b=BLOC))
                        wcols = sepw_c[i]
                        acc = cst([128, BLOC, T])
                        nc.vector.tensor_scalar(out=acc[:], in0=pad[:, :, 0:T],
                                                scalar1=wcols[0][:, m:m + 1],
                                                scalar2=None, op0=ALU.mult)
                        nc.vector.scalar_tensor_tensor(
                            out=acc[:], in0=pad[:, :, d:T + d],
                            scalar=wcols[1][:, m:m + 1], in1=acc[:],
                            op0=ALU.mult, op1=ALU.add)
                        a = cst([128, FREE], f32r)
                        nc.vector.scalar_tensor_tensor(
                            out=a[:].rearrange("c (b t) -> c b t", b=BLOC),
                            in0=pad[:, :, 2 * d:T + 2 * d],
                            scalar=wcols[2][:, m:m + 1],
                            in1=acc[:], op0=ALU.mult, op1=ALU.add)
                        a_t.append(a)

                    y1_t = layer_norm_gelu(a_t, sepb_c[i], bm_c[0:1, i:i + 1],
                                           n1g_c[i], n1b_c[i], f32r)

                    y2_t = []
                    for mo in range(2):
                        pm = pool_ps.tile([128, FREE], f32, tag="bigps")
                        for n in range(NSUB):
                            sl = slice(n * 512, (n + 1) * 512)
                            for k in range(2):
                                nc.tensor.matmul(
                                    pm[:, sl],
                                    wc11_r[i][k][:, mo * 128:(mo + 1) * 128],
                                    y1_t[k][:, sl], start=(k == 0), stop=(k == 1))
                        y2 = cst([128, FREE], f32r)
                        nc.scalar.activation(y2[:], pm[:], AF.Copy,
                                             bias=0.0, scale=1.0)
                        y2_t.append(y2)

                    y3_t = layer_norm_gelu(y2_t, c11b_c[i],
                                           bm_c[0:1, NL + i:NL + i + 1],
                                           n2g_c[i], n2b_c[i], f32)

                    for m in range(2):
                        nc.vector.tensor_tensor(out=h_t[m][:], in0=h_t[m][:],
                                                in1=y3_t[m][:], op=ALU.add)

            # ================= spline phase =================
            with tc.tile_pool(name="spl", bufs=1) as sp, \
                 tc.tile_pool(name="sing", bufs=24) as sgp, \
                 tc.tile_pool(name="msk", bufs=12) as mp, \
                 tc.tile_pool(name="spp", bufs=4, space="PSUM") as pool_pp, \
                 tc.tile_pool(name="ldp", bufs=2, space="PSUM") as pool_pl:

                x1_2 = x1_f[:].rearrange("c b t -> c (b t)")

                _sctr = [0]

                def sing(dty=f32):
                    _sctr[0] += 1
                    return sgp.tile([HALF, CH], dty, tag="ss",
                                    name=f"ss{_sctr[0]}")

                def msk():
                    _sctr[0] += 1
                    return mp.tile([HALF, CH], u8, tag="mm",
                                   name=f"mm{_sctr[0]}")

                def stt(o, i0, s, i1, op0, op1):
                    nc.vector.scalar_tensor_tensor(out=o, in0=i0, scalar=s, in1=i1,
                                                   op0=op0, op1=op1)

                def tt(o, i0, i1, op):
                    nc.vector.tensor_tensor(out=o, in0=i0, in1=i1, op=op)

                def ts(o, i0, s1, s2, op0, op1=None):
                    if op1 is None:
                        nc.vector.tensor_scalar(out=o, in0=i0, scalar1=s1,
                                                scalar2=None, op0=op0)
                    else:
                        nc.vector.tensor_scalar(out=o, in0=i0, scalar1=s1,
                                                scalar2=s2, op0=op0, op1=op1)

                for cc in range(NCH):
                    sl = slice(cc * CH, (cc + 1) * CH)

                    def proj_plane(j):
                        pm = pool_pp.tile([HALF, CH], f32, tag="plps")
                        for k in range(2):
                            nc.tensor.matmul(
                                pm[:], wproj_r[k][:, j * HALF:(j + 1) * HALF],
                                h_t[k][:, sl], start=(k == 0), stop=(k == 1))
                        return pm

                    def ingest(j, tag, nm):
                        pm = proj_plane(j)
                        e_ = sp.tile([HALF, CH], f32, tag=tag, name=nm)
                        nc.scalar.activation(e_[:], pm[:], AF.Exp,
                                             bias=pjb_c[:, j:j + 1], scale=1.0)
                        return e_

                    # ---- widths ----
                    ew = [ingest(j, f"e{j}", f"ew{cc}_{j}") for j in range(NB)]
                    Fw = [None] * (NB + 1)
                    Fw[1] = ew[0]
                    for k in range(2, NB + 1):
                        f_ = sp.tile([HALF, CH], f32, tag=f"F{k}")
                        tt(f_[:], Fw[k - 1][:], ew[k - 1][:], ALU.add)
                        Fw[k] = f_
                    Pw = sing()
                    ts(Pw[:], Fw[NB][:], 0.001 / CC, None, ALU.mult)
                    Sw = sing()
                    ts(Sw[:], Fw[NB][:], 1.0 / (10.0 * CC), None, ALU.mult)
                    xc = sing()
                    ts(xc[:], x1_2[:, sl], -TB, TB, ALU.max, ALU.min)
                    uS = sing()
                    stt(uS[:], xc[:], TB, Sw[:], ALU.add, ALU.mult)
                    Fk = [None] * NB
                    f1 = sp.tile([HALF, CH], f32, tag="F1")
                    stt(f1[:], Pw[:], 1.0, ew[0][:], ALU.mult, ALU.add)
                    Fk[1] = f1
                    for k in range(2, NB):
                        stt(Fw[k][:], Pw[:], float(k), Fw[k][:], ALU.mult, ALU.add)
                        Fk[k] = Fw[k]
                    mk = []
                    for k in range(1, NB):
                        m_ = msk()
                        tt(m_[:], uS[:], Fk[k][:], ALU.is_ge)
                        mk.append(m_)
                    selF = sing()
                    nc.gpsimd.memset(selF[:], 0.0)
                    for k in range(1, NB):
                        nc.vector.copy_predicated(selF[:], mk[k - 1][:], Fk[k][:])
                    sele = sing()
                    nc.scalar.activation(sele[:], ew[0][:], AF.Copy,
                                         bias=0.0, scale=1.0)
                    for k in range(1, NB):
                        nc.vector.copy_predicated(sele[:], mk[k - 1][:], ew[k][:])
                    denw = sing()
                    tt(denw[:], sele[:], Pw[:], ALU.add)
                    invdw = sing()
                    nc.vector.reciprocal(invdw[:], denw[:])
                    th = sing()
                    tt(th[:], uS[:], selF[:], ALU.subtract)
                    tt(th[:], th[:], invdw[:], ALU.mult)

                    # ---- heights (reuse e/F tags) ----
                    eh = [ingest(NB + j, f"e{j}", f"ehh{cc}_{j}") for j in range(NB)]
                    Fh = [None] * (NB + 1)
                    Fh[1] = eh[0]
                    for k in range(2, NB + 1):
                        f_ = sp.tile([HALF, CH], f32, tag=f"F{k}")
                        tt(f_[:], Fh[k - 1][:], eh[k - 1][:], ALU.add)
                        Fh[k] = f_
                    Ph = sing()
                    ts(Ph[:], Fh[NB][:], 0.001 / CC, None, ALU.mult)
                    invSh = sing()
                    nc.vector.reciprocal(invSh[:], Fh[NB][:])
                    Hk = [None] * NB
                    h1_ = sp.tile([HALF, CH], f32, tag="F1")
                    stt(h1_[:], Ph[:], 1.0, eh[0][:], ALU.mult, ALU.add)
                    Hk[1] = h1_
                    for k in range(2, NB):
                        stt(Fh[k][:], Ph[:], float(k), Fh[k][:], ALU.mult, ALU.add)
                        Hk[k] = Fh[k]
                    selH = sing()
                    nc.gpsimd.memset(selH[:], 0.0)
                    for k in range(1, NB):
                        nc.vector.copy_predicated(selH[:], mk[k - 1][:], Hk[k][:])
                    selh2 = sing()
                    nc.scalar.activation(selh2[:], eh[0][:], AF.Copy,
                                         bias=0.0, scale=1.0)
                    for k in range(1, NB):
                        nc.vector.copy_predicated(selh2[:], mk[k - 1][:], eh[k][:])
                    ichp = sing()
                    stt(ichp[:], selH[:], 10.0 * CC, invSh[:], ALU.mult, ALU.mult)
                    ih = sing()
                    stt(ih[:], selh2[:], 10.0 * CC, invSh[:], ALU.mult, ALU.mult)
                    ts(ih[:], ih[:], 10.0 * MINW, None, ALU.add)

                    # ---- derivatives ----
                    dts = []
                    for k in range(NB - 1):
                        j = 2 * NB + k
                        pm = proj_plane(j)
                        ed = sing()
                        nc.scalar.activation(ed[:], pm[:], AF.Exp,
                                             bias=pjb_c[:, j:j + 1], scale=1.0)
                        dt_ = sp.tile([HALF, CH], f32, tag=f"dt{k}")
                        nc.scalar.activation(dt_[:], ed[:], AF.Ln,
                                             bias=one_c[:HALF, :], scale=1.0)
                        dts.append(dt_)
                    ider = sing()
                    nc.gpsimd.memset(ider[:], 1.0 - MIND)
                    for k in range(1, NB):
                        nc.vector.copy_predicated(ider[:], mk[k - 1][:],
                                                  dts[k - 1][:])
                    iderp = sing()
                    nc.scalar.activation(iderp[:], dts[0][:], AF.Copy,
                                         bias=0.0, scale=1.0)
                    for k in range(1, NB - 1):
                        nc.vector.copy_predicated(iderp[:], mk[k - 1][:], dts[k][:])
                    nc.vector.copy_predicated(iderp[:], mk[NB - 2][:], cone_d[:])
                    ts(ider[:], ider[:], MIND, None, ALU.add)
                    ts(iderp[:], iderp[:], MIND, None, ALU.add)

                    # ---- rational-quadratic evaluation ----
                    idl = sing()
                    tt(idl[:], ih[:], Sw[:], ALU.mult)
                    tt(idl[:], idl[:], invdw[:], ALU.mult)
                    th2 = sing()
                    nc.scalar.activation(th2[:], th[:], AF.Square,
                                         bias=0.0, scale=1.0)
                    tt_ = sing()
                    tt(tt_[:], th[:], th2[:], ALU.subtract)
                    a1 = sing()
                    tt(a1[:], idl[:], th2[:], ALU.mult)
                    a2 = sing()
                    tt(a2[:], ider[:], tt_[:], ALU.mult)
                    tt(a1[:], a1[:], a2[:], ALU.add)
                    tt(a1[:], a1[:], ih[:], ALU.mult)          # num
                    s1 = sing()
                    tt(s1[:], ider[:], iderp[:], ALU.add)
                    stt(s1[:], idl[:], -2.0, s1[:], ALU.mult, ALU.add)
                    tt(s1[:], s1[:], tt_[:], ALU.mult)
                    den = sing()
                    tt(den[:], s1[:], idl[:], ALU.add)
                    invden = sing()
                    nc.vector.reciprocal(invden[:], den[:])
                    xo = sing()
                    tt(xo[:], a1[:], invden[:], ALU.mult)
                    tt(xo[:], xo[:], ichp[:], ALU.add)
                    ts(xo[:], xo[:], -TB, None, ALU.add)
                    b1 = sing()
                    tt(b1[:], iderp[:], th2[:], ALU.mult)
                    b2 = sing()
                    stt(b2[:], idl[:], 2.0, tt_[:], ALU.mult, ALU.mult)
                    tt(b1[:], b1[:], b2[:], ALU.add)
                    om2 = sing()
                    nc.scalar.activation(om2[:], th[:], AF.Square,
                                         bias=one_c[:HALF, :], scale=-1.0)
                    tt(om2[:], om2[:], ider[:], ALU.mult)
                    tt(b1[:], b1[:], om2[:], ALU.add)
                    idl2 = sing()
                    nc.scalar.activation(idl2[:], idl[:], AF.Square,
                                         bias=0.0, scale=1.0)
                    tt(b1[:], b1[:], idl2[:], ALU.mult)
                    lnd = sing()
                    nc.scalar.activation(lnd[:], b1[:], AF.Ln, bias=0.0, scale=1.0)
                    lnden = sing()
                    nc.scalar.activation(lnden[:], den[:], AF.Ln, bias=0.0, scale=1.0)
                    lad = sing()
                    stt(lad[:], lnden[:], -2.0, lnd[:], ALU.mult, ALU.add)

                    o1 = msk()
                    ts(o1[:], x1_2[:, sl], TB, None, ALU.is_gt)
                    o2 = msk()
                    ts(o2[:], x1_2[:, sl], -TB, None, ALU.is_lt)
                    nc.vector.copy_predicated(xo[:], o1[:], x1_2[:, sl])
                    nc.vector.copy_predicated(xo[:], o2[:], x1_2[:, sl])
                    nc.vector.copy_predicated(lad[:], o1[:], czero_f[:])
                    nc.vector.copy_predicated(lad[:], o2[:], czero_f[:])

                    lad_r = sing(f32r)
                    nc.vector.tensor_copy(lad_r[:], lad[:])
                    ldps = pool_pl.tile([1, CH], f32, tag="ldps")
                    nc.tensor.matmul(ldps[:], ones_r[:HALF, :], lad_r[:],
                                     start=True, stop=True)
                    nc.scalar.activation(ldsb[:, sl], ldps[:], AF.Copy,
                                         bias=0.0, scale=1.0)

                    b_i, t_i = cc // (NCH // BLOC), (cc % (NCH // BLOC)) * CH
                    nc.sync.dma_start(
                        xout_d[b_i:b_i + 1, HALF:IN_CH,
                               t_i:t_i + CH].rearrange("b c t -> c (b t)"),
                        xo[:])

            ld_t = cp.tile([1, BLOC], f32, tag="ldt")
            nc.vector.tensor_reduce(
                out=ld_t[:], in_=ldsb[:].rearrange("p (b t) -> p b t", b=BLOC),
                axis=mybir.AxisListType.X, op=ALU.add)
            nc.sync.dma_start(ld_d[:], ld_t[:])

    nc.compile()
    return nc


def _prep_shared(inputs):
    f = np.float32
    shared = {}
    shared["wpre"] = np.ascontiguousarray(np.asarray(inputs["pre_w"], f).T)
    shared["preb"] = np.asarray(inputs["pre_b"], f)
    shared["wc11"] = np.ascontiguousarray(
        np.transpose(np.asarray(inputs["c11_w"], f), (0, 2, 1)))
    shared["c11b"] = np.asarray(inputs["c11_b"], f)
    shared["sepw"] = np.ascontiguousarray(
        np.transpose(np.asarray(inputs["sep_w"], f)[:, :, 0, :], (0, 2, 1)))
    shared["sepb"] = np.asarray(inputs["sep_b"], f)
    shared["n1g"] = np.asarray(inputs["n1_g"], f)
    shared["n1b"] = np.asarray(inputs["n1_b"], f)
    shared["n2g"] = np.asarray(inputs["n2_g"], f)
    shared["n2b"] = np.asarray(inputs["n2_b"], f)
    bm = np.stack([
        np.asarray(inputs["sep_b"], np.float64).mean(1),
        np.asarray(inputs["c11_b"], np.float64).mean(1)]).astype(f)
    shared["bmeans"] = np.ascontiguousarray(bm)
    pw = np.asarray(inputs["proj_w"], np.float64).reshape(HALF, NPAR, FILT)
    pb = np.asarray(inputs["proj_b"], np.float64).reshape(HALF, NPAR)
    scal = np.ones((NPAR,)); scal[:2 * NB] = INV_SQRT_FILT
    pwj = np.transpose(pw, (1, 0, 2)) * scal[:, None, None]
    pbj = np.transpose(pb, (1, 0)) * scal[:, None]
    shared["wproj"] = np.ascontiguousarray(
        pwj.reshape(NPAR * HALF, FILT).T.astype(f))
    bias = pbj.copy()
    bias[:2 * NB] += LN10C
    shared["pjb"] = np.ascontiguousarray(bias.astype(f))
    return shared


def kernel(**inputs):
    if "nc" not in _CACHE:
        _CACHE["nc"] = _build_program()
    nc = _CACHE["nc"]

    x = np.asarray(inputs["x"], dtype=np.float32)
    shared = _prep_shared(inputs)

    in_maps = []
    for c in range(NCORES):
        m = dict(shared)
        m["x"] = np.ascontiguousarray(x[c * BLOC:(c + 1) * BLOC])
        in_maps.append(m)

    res = run_bass_kernel_spmd(nc, in_maps, core_ids=list(range(NCORES)))
    x_out = np.empty((B, IN_CH, T), np.float32)
    logdet = np.empty((B,), np.float32)
    for c in range(NCORES):
        r = res.results[c]
        x_out[c * BLOC:(c + 1) * BLOC] = r["xout"]
        logdet[c * BLOC:(c + 1) * BLOC] = r["ld"][0]
    return x_out, logdet
